# revision 7
# baseline (speedup 1.0000x reference)
"""Multi-head attention (query-axis softmax variant) on 8 Trainium2 NeuronCores.

Problem: B=4, T=2048, C=1024, H=16, Dh=64.
  q/k/v = per-head projections of x; wei = (q k^T) * C**-0.5, causal-masked;
  softmax over the QUERY axis (axis=2 of (B,H,T,S)); out = attn @ v, concat
  heads, project with Wp and add bp.

Sharding: 8 cores = 4 batches x 2 head-groups (8 heads each).  Each core
computes a partial projection output for its batch; host sums the two
group partials per batch and adds the bias.

Per-core dataflow is fully "transposed" (features on partitions, tokens on
the free axis) so the query-axis softmax stats become free-axis reductions:
  xT (C, T) -> qT/kT per head-pair (128, T) -> scores W[s,t] per 128-s tile
  -> P = exp(W*scale), causal mask via -1e30 psum add (narrow tiles) or a
  post-exp 0/1 multiply on gpsimd (wide tiles) -> Z row sums (ACT accum for
  narrow tiles, DVE reduce for wide) -> vp = v/Z -> attout accumulated per
  512-col chunk over retained P rows -> y = attout^T.T @ WpT.

Engine plan: homogeneous PE bursts (score pairs in disjoint row groups, AV
pairs in disjoint col groups, projection c-tile chains) keep LDWEIGHTS
overlapped; exp runs as merged two-head 3D-AP ACTIVATEs for wide tiles and
per-head accum ACTIVATEs for narrow ones; fillers (v/qk/output projections)
are interleaved between score blocks to cover the exp shadows.
"""
import numpy as np

T = 2048
C = 1024
H = 16
DH = 64
B = 4
SCALE = float(C) ** -0.5
NEG = -1e30
P = 128
NCT = 8       # c-tiles (contraction tiles of 128 over C)
NST = 16      # s-tiles of 128 over T
MERGE_MAX = 8  # tiles i < MERGE_MAX use merged-head exp + DVE Z; rest ACT accum

_CACHE = {}


def _build_nc():
    import concourse.bacc as bacc
    import concourse.tile as tile
    import concourse.mybir as mybir

    FP = mybir.dt.float32
    BF = mybir.dt.bfloat16
    AX = mybir.AxisListType.X
    EXP = mybir.ActivationFunctionType.Exp

    nc = bacc.Bacc("TRN2", target_bir_lowering=False, debug=False, num_devices=8)

    xT_d = nc.declare_dram_parameter("xt", [C, T], BF, isOutput=False)
    wq_d = nc.declare_dram_parameter("wq", [C, 512], BF, isOutput=False)
    wk_d = nc.declare_dram_parameter("wk", [C, 512], BF, isOutput=False)
    wv_d = nc.declare_dram_parameter("wv", [C, 512], BF, isOutput=False)
    wp_d = nc.declare_dram_parameter("wpt", [512, C], BF, isOutput=False)
    trif_d = nc.declare_dram_parameter("trif", [P, 256], FP, isOutput=False)
    y_d = nc.declare_dram_parameter("y", [T, C], BF, isOutput=True)

    def blocks(i):
        t0 = P * i
        return [(t0, 1024), (1024, 2048)] if i < 8 else [(t0, 2048)]

    with tile.TileContext(nc) as tc:
        with (
            tc.tile_pool(name="perm", bufs=1) as perm,
            tc.tile_pool(name="wst", bufs=2) as wst,
            tc.tile_pool(name="prp", bufs=1) as prp,
            tc.tile_pool(name="st", bufs=4) as st,
            tc.tile_pool(name="ytp", bufs=2) as ytp,
            tc.tile_pool(name="scp", bufs=1, space="PSUM") as scp,
            tc.tile_pool(name="gp", bufs=2, space="PSUM") as gp,
        ):
            trif = perm.tile([P, 2, 128], FP, tag="trif")
            nc.sync.dma_start(trif[:], trif_d.ap().rearrange("p (a c) -> p a c", a=2))

            warm = perm.tile([P, 512], BF, tag="warm")
            nc.vector.memset(warm[:], 0.0)
            dummy = perm.tile([P, 1], BF, tag="dummy")
            # preload the exp table set off the critical path
            nc.scalar.activation(dummy[:], warm[:, 0:1], EXP, scale=1.0)

            # --- staged weights for pair 0 + full xT (per-ctile DMAs) ---
            wqt = {}
            wkt = {}

            def stage_w(pn):
                wqt[pn] = wst.tile([P, NCT, P], BF, tag="wq", name=f"wq{pn}")
                wkt[pn] = wst.tile([P, NCT, P], BF, tag="wk", name=f"wk{pn}")
                nc.sync.dma_start(
                    wqt[pn][:],
                    wq_d.ap()[:, P * pn:P * pn + P].rearrange("(a c) m -> c a m", c=P))
                nc.sync.dma_start(
                    wkt[pn][:],
                    wk_d.ap()[:, P * pn:P * pn + P].rearrange("(a c) m -> c a m", c=P))

            stage_w(0)
            xT = perm.tile([P, NCT, T], BF, tag="xT")
            for ct in range(NCT):
                nc.sync.dma_start(
                    xT[:, ct, :],
                    xT_d.ap()[P * ct:P * ct + P, :])
            wv = perm.tile([P, NCT, 512], BF, tag="wv")
            nc.sync.dma_start(wv[:], wv_d.ap().rearrange("(a c) m -> c a m", c=P))
            wpt = perm.tile([P, 4, C], BF, tag="wpt")
            nc.sync.dma_start(wpt[:], wp_d.ap().rearrange("(a c) m -> c a m", c=P))

            q_sl = perm.tile([P, 2, T], BF, tag="q")
            k_sl = perm.tile([P, 2, T], BF, tag="k")
            v_sb = perm.tile([P, NST, 512], BF, tag="v")
            ao = perm.tile([P, 4, T], BF, tag="ao")
            vp = perm.tile([P, 2, NST, 128], BF, tag="vp")

            # --- warmup junk (PE clock gate) while first DMAs land ---
            sck = scp.tile([P, 2, 1024], FP, tag="sc", name="sck_prefix")
            for _ in range(12):
                nc.tensor.matmul(sck[:, 0, 0:512], lhsT=warm[:, :P], rhs=warm[:],
                                 start=True, stop=True)

            # --- prefix: q/k projections of pair 0, ct-outer over 8 banks ---
            gA = gp.tile([P, 1024], FP, tag="g", name="pre_qA")
            gB = gp.tile([P, 1024], FP, tag="g", name="pre_qB")
            for ct in range(NCT):
                for gi, gt in ((0, gA), (1, gB)):
                    for h in range(2):
                        nc.tensor.matmul(
                            gt[:, 512 * h:512 * h + 512],
                            lhsT=wqt[0][:, ct, :],
                            rhs=xT[:, ct, 1024 * gi + 512 * h:1024 * gi + 512 * h + 512],
                            start=(ct == 0), stop=(ct == NCT - 1))
                for gi in range(2):
                    for h in range(2):
                        nc.tensor.matmul(
                            sck[:, gi, 512 * h:512 * h + 512],
                            lhsT=wkt[0][:, ct, :],
                            rhs=xT[:, ct, 1024 * gi + 512 * h:1024 * gi + 512 * h + 512],
                            start=(ct == 0), stop=(ct == NCT - 1))
            nc.vector.tensor_copy(q_sl[:, 0, 0:1024], gA[:])
            nc.vector.tensor_copy(q_sl[:, 0, 1024:2048], gB[:])
            nc.vector.tensor_copy(k_sl[:, 0, 0:1024], sck[:, 0, :])
            nc.vector.tensor_copy(k_sl[:, 0, 1024:2048], sck[:, 1, :])

            # --- filler closures ---
            def qk_group(pn, which, gi):
                def emit():
                    gt = gp.tile([P, 1024], FP, tag="g", name=f"{which}{pn}_{gi}")
                    wt = wqt[pn] if which == "q" else wkt[pn]
                    dst = q_sl if which == "q" else k_sl
                    for ct in range(NCT):
                        for h in range(2):
                            nc.tensor.matmul(
                                gt[:, 512 * h:512 * h + 512],
                                lhsT=wt[:, ct, :],
                                rhs=xT[:, ct, 1024 * gi + 512 * h:1024 * gi + 512 * h + 512],
                                start=(ct == 0), stop=(ct == NCT - 1))
                    nc.vector.tensor_copy(dst[:, pn % 2, 1024 * gi:1024 * gi + 1024], gt[:])
                return emit

            def v_group(g):
                def emit():
                    gt = gp.tile([P, 1024], FP, tag="g", name=f"v{g}")
                    for ct in range(NCT):
                        for h in range(2):
                            nc.tensor.matmul(
                                gt[:, 512 * h:512 * h + 512],
                                lhsT=xT[:, ct, P * (2 * g + h):P * (2 * g + h) + P],
                                rhs=wv[:, ct, :],
                                start=(ct == 0), stop=(ct == NCT - 1))
                    nc.vector.tensor_copy(v_sb[:, 2 * g:2 * g + 2, :], gt[:])
                return emit

            def proj_group(tt):
                def emit():
                    gt = gp.tile([P, 1024], FP, tag="g", name=f"pj{tt}")
                    for pp in range(4):
                        for h in range(2):
                            nc.tensor.matmul(
                                gt[:, 512 * h:512 * h + 512],
                                lhsT=ao[:, pp, P * tt:P * tt + P],
                                rhs=wpt[:, pp, 512 * h:512 * h + 512],
                                start=(pp == 0), stop=(pp == 3))
                    yt = ytp.tile([P, 1024], BF, tag="yt", name=f"yt{tt}")
                    nc.vector.tensor_copy(yt[:], gt[:])
                    nc.sync.dma_start(y_d.ap()[P * tt:P * tt + P, :], yt[:])
                return emit

            # --- attention pairs ---
            for p in range(4):
                sl = p % 2
                if p == 0:
                    fill = [v_group(g) for g in range(8)] \
                        + [qk_group(1, w, gi) for w in ("q", "k") for gi in range(2)]
                    stage_w(1)
                elif p < 3:
                    fill = [qk_group(p + 1, w, gi) for w in ("q", "k") for gi in range(2)]
                    stage_w(p + 1)
                else:
                    fill = []
                # pop cadence: one filler every other score block spreads the
                # pair's fillers across its exp shadows; junk matmuls keep the
                # PE's HAM clock gate warm once real fillers run dry
                bcount = 0

                def pop_fill():
                    if fill:
                        fill.pop(0)()
                    else:
                        gt = gp.tile([P, 1024], FP, tag="g",
                                     name=f"junk{p}_{bcount}")
                        for _ in range(4):
                            nc.tensor.matmul(gt[:, 0:512], lhsT=warm[:, :P],
                                             rhs=warm[:], start=True, stop=True)

                def emit_av(c):
                    gt = gp.tile([P, 1024], FP, tag="g", name=f"av{p}_{c}")
                    jmax = 4 * c + 3
                    for j in range(jmax + 1):
                        lo2 = max(512 * c, P * j)
                        for hl in range(2):
                            nc.tensor.matmul(
                                gt[64 * hl:64 * hl + 64, lo2 - 512 * c:512],
                                lhsT=vp[:, sl, j, 64 * hl:64 * hl + 64],
                                rhs=prow[j][:, hl, lo2 - P * j:512 * c + 512 - P * j],
                                start=(j == 0), stop=(j == jmax))
                    nc.vector.tensor_copy(ao[:, p, 512 * c:512 * c + 512], gt[:, 0:512])
                    if p == 3:
                        for tt in range(4 * c, 4 * c + 4):
                            fill.append(proj_group(tt))

                prow = {}
                for i in range(NST):
                    t0 = P * i
                    w_i = T - t0
                    prow[i] = prp.tile([P, 2, w_i], BF, tag=f"pr{i}",
                                       name=f"pr{p}_{i}")
                    z = st.tile([P, 2], FP, tag="z", name=f"z{p}_{i}")
                    zs = st.tile([P, 2], FP, tag="zs", name=f"zs{p}_{i}")
                    rz = st.tile([P, 2], FP, tag="rz", name=f"rz{p}_{i}")
                    nb = len(blocks(i))
                    for b, (lo, hi) in enumerate(blocks(i)):
                        bw = hi - lo
                        sc = scp.tile([P, 2, 1024], FP, tag="sc", name=f"sc{p}_{i}_{b}")
                        for clo in range(0, bw, 512):
                            chi = min(clo + 512, bw)
                            for hl in range(2):
                                hb = 64 * hl
                                nc.tensor.matmul(
                                    sc[:, hl, clo:chi],
                                    lhsT=k_sl[hb:hb + 64, sl, t0:t0 + P],
                                    rhs=q_sl[hb:hb + 64, sl, lo + clo:lo + chi],
                                    start=True, stop=True)
                        if b == 0:
                            # causal mask: additive -1e30 triangle on the
                            # diagonal 128-block (block 0 always starts at t0)
                            nc.vector.tensor_add(sc[:, :, 0:128], sc[:, :, 0:128],
                                                 trif[:])
                        if i >= MERGE_MAX:
                            # narrow tile: per-head exp, Z via ACT accumulator
                            for hl in range(2):
                                nc.scalar.activation(
                                    prow[i][:, hl, :], sc[:, hl, 0:bw], EXP,
                                    scale=SCALE, accum_out=z[:, hl:hl + 1])
                        else:
                            # wide tile: merged two-head exp; accumulator gives
                            # zh0+zh1 per partition (same s for both heads)
                            nc.scalar.activation(
                                prow[i][:, :, lo - t0:hi - t0], sc[:, :, 0:bw],
                                EXP, scale=SCALE, accum_out=zs[:, b:b + 1])
                        if bcount % 2 == 0:
                            pop_fill()
                        bcount += 1
                    if i < MERGE_MAX:
                        # z_h0 by DVE reduce; z_h1 = (zh0+zh1) - z_h0
                        nc.vector.reduce_sum(z[:, 0:1], prow[i][:, 0, :], axis=AX)
                        if nb == 2:
                            nc.vector.tensor_add(zs[:, 0:1], zs[:, 0:1], zs[:, 1:2])
                        nc.vector.tensor_sub(z[:, 1:2], zs[:, 0:1], z[:, 0:1])
                    nc.vector.reciprocal(rz[:], z[:])
                    for hl in range(2):
                        nc.vector.tensor_scalar_mul(
                            vp[:, sl, i, 64 * hl:64 * hl + 64],
                            v_sb[:, i, 128 * p + 64 * hl:128 * p + 64 * hl + 64],
                            rz[:, hl:hl + 1])
                    if i in (4, 8, 12):
                        emit_av(i // 4 - 1)
                emit_av(3)
                for f in fill:
                    f()

    nc.compile()
    return nc


def _get_nc():
    if "nc" not in _CACHE:
        _CACHE["nc"] = _build_nc()
    return _CACHE["nc"]


def _in_maps(x, Wq, Wk, Wv, Wp):
    import ml_dtypes
    trif = np.tril(np.full((P, P), NEG, np.float32), -1)
    trif2 = np.concatenate([trif, trif], 1)
    maps = []
    for b in range(B):
        xT = np.ascontiguousarray(x[b].T)
        for g in range(2):
            heads = range(8 * g, 8 * g + 8)
            maps.append({
                "xt": xT.astype(ml_dtypes.bfloat16),
                "wq": np.ascontiguousarray(np.concatenate([Wq[h] for h in heads], 1)).astype(ml_dtypes.bfloat16),
                "wk": np.ascontiguousarray(np.concatenate([Wk[h] for h in heads], 1)).astype(ml_dtypes.bfloat16),
                "wv": np.ascontiguousarray(np.concatenate([Wv[h] for h in heads], 1)).astype(ml_dtypes.bfloat16),
                "wpt": np.ascontiguousarray(Wp[:, 512 * g:512 * g + 512].T).astype(ml_dtypes.bfloat16),
                "trif": np.ascontiguousarray(trif2),
            })
    return maps


def kernel(x, Wq, Wk, Wv, Wp, bp):
    from concourse.bass_utils import run_bass_kernel_spmd

    x = np.asarray(x, np.float32)
    Wq = np.asarray(Wq, np.float32)
    Wk = np.asarray(Wk, np.float32)
    Wv = np.asarray(Wv, np.float32)
    Wp = np.asarray(Wp, np.float32)
    bp = np.asarray(bp, np.float32)

    nc = _get_nc()
    res = run_bass_kernel_spmd(nc, _in_maps(x, Wq, Wk, Wv, Wp), list(range(8)))
    y = np.empty((B, T, C), np.float32)
    for b in range(B):
        y[b] = (res.results[2 * b]["y"].astype(np.float32)
                + res.results[2 * b + 1]["y"].astype(np.float32) + bp)
    return y


# revision 10
# speedup vs baseline: 1.1073x; 1.1073x over previous
"""Multi-head attention (query-axis softmax variant) on 8 Trainium2 NeuronCores.

Problem: B=4, T=2048, C=1024, H=16, Dh=64.
  q/k/v = per-head projections of x; wei = (q k^T) * C**-0.5, causal-masked;
  softmax over the QUERY axis (axis=2 of (B,H,T,S)); out = attn @ v, concat
  heads, project with Wp and add bp.

Sharding: 8 cores = 4 batches x 2 head-groups (8 heads each).  Each core
computes a partial projection output for its batch; host sums the two
group partials per batch and adds the bias.

Per-core dataflow is fully "transposed" (features on partitions, tokens on
the free axis) so the query-axis softmax stats become free-axis reductions.
Score blocks rotate through pipelined PSUM slots (A = [P,2,1024], its second
half, and B = [P,2,512]) so the next block's matmuls run while the previous
block's exp drains.  Exps are merged two-head 3D-AP ACTIVATEs whose
accumulator yields zh0+zh1 per partition (both heads share the same s on a
partition); h0's Z comes from one DVE reduce of the retained P row and h1's
by subtraction on gpsimd.  attout accumulates per 512-col chunk over the
retained P rows with col-group-paired matmuls; projections (v, q/k of the
next pair, output) fill the PE between score bursts.
"""
import numpy as np

T = 2048
C = 1024
H = 16
DH = 64
B = 4
SCALE = float(C) ** -0.5
NEG = -1e30
P = 128
NCT = 8       # c-tiles (contraction tiles of 128 over C)
NST = 16      # s-tiles of 128 over T

_CACHE = {}


def _build_nc():
    import concourse.bacc as bacc
    import concourse.tile as tile
    import concourse.mybir as mybir

    FP = mybir.dt.float32
    BF = mybir.dt.bfloat16
    AX = mybir.AxisListType.X
    EXP = mybir.ActivationFunctionType.Exp

    nc = bacc.Bacc("TRN2", target_bir_lowering=False, debug=False, num_devices=8)

    xT_d = nc.declare_dram_parameter("xt", [C, T], BF, isOutput=False)
    wq_d = nc.declare_dram_parameter("wq", [C, 512], BF, isOutput=False)
    wk_d = nc.declare_dram_parameter("wk", [C, 512], BF, isOutput=False)
    wv_d = nc.declare_dram_parameter("wv", [C, 512], BF, isOutput=False)
    wp_d = nc.declare_dram_parameter("wpt", [512, C], BF, isOutput=False)
    trif_d = nc.declare_dram_parameter("trif", [P, 256], FP, isOutput=False)
    y_d = nc.declare_dram_parameter("y", [T, C], BF, isOutput=True)

    def blocks(i):
        # (lo, hi, slot): slot 0 = A[:, :, 0:1024], 1 = A[:, :, 512:1024],
        # 2 = B.  Rotation gives pipeline depth >= 2 between exp and the next
        # block's matmuls.
        t0 = P * i
        if i < 8:
            return [(t0, 1024, 0), (1024, 1536, 2), (1536, 2048, 1)]
        if i < 12:
            return [(t0, 2048, 0)]
        return [(t0, 2048, 2)]

    with tile.TileContext(nc) as tc:
        with (
            tc.tile_pool(name="perm", bufs=1) as perm,
            tc.tile_pool(name="wst", bufs=2) as wst,
            tc.tile_pool(name="prp", bufs=1) as prp,
            tc.tile_pool(name="st", bufs=6) as st,
            tc.tile_pool(name="ytp", bufs=2) as ytp,
            tc.tile_pool(name="sca", bufs=1, space="PSUM") as sca,
            tc.tile_pool(name="scb", bufs=1, space="PSUM") as scb,
            tc.tile_pool(name="gp", bufs=2, space="PSUM") as gp,
        ):
            trif = perm.tile([P, 2, 128], FP, tag="trif")
            nc.sync.dma_start(trif[:], trif_d.ap().rearrange("p (a c) -> p a c", a=2))

            warm = perm.tile([P, 512], BF, tag="warm")
            nc.vector.memset(warm[:], 0.0)
            dummy = perm.tile([P, 1], BF, tag="dummy")
            # preload the exp table set off the critical path
            nc.scalar.activation(dummy[:], warm[:, 0:1], EXP, scale=1.0)

            wqt = {}
            wkt = {}

            def stage_w(pn):
                wqt[pn] = wst.tile([P, NCT, P], BF, tag="wq", name=f"wq{pn}")
                wkt[pn] = wst.tile([P, NCT, P], BF, tag="wk", name=f"wk{pn}")
                nc.sync.dma_start(
                    wqt[pn][:],
                    wq_d.ap()[:, P * pn:P * pn + P].rearrange("(a c) m -> c a m", c=P))
                nc.sync.dma_start(
                    wkt[pn][:],
                    wk_d.ap()[:, P * pn:P * pn + P].rearrange("(a c) m -> c a m", c=P))

            stage_w(0)
            xT = perm.tile([P, NCT, T], BF, tag="xT")
            for ct in range(NCT):
                nc.sync.dma_start(xT[:, ct, :], xT_d.ap()[P * ct:P * ct + P, :])
            wv = perm.tile([P, NCT, 512], BF, tag="wv")
            nc.sync.dma_start(wv[:], wv_d.ap().rearrange("(a c) m -> c a m", c=P))
            wpt = perm.tile([P, 4, C], BF, tag="wpt")
            nc.sync.dma_start(wpt[:], wp_d.ap().rearrange("(a c) m -> c a m", c=P))

            q_sl = perm.tile([P, 2, T], BF, tag="q")
            k_sl = perm.tile([P, 2, T], BF, tag="k")
            v_sb = perm.tile([P, NST, 512], BF, tag="v")
            ao = perm.tile([P, 4, T], BF, tag="ao")
            vp = perm.tile([P, 2, NST, 128], BF, tag="vp")

            # score slots: allocated once, rotation handled by subtile deps
            scA = sca.tile([P, 2, 1024], FP, tag="sca", name="scA")
            scB = scb.tile([P, 2, 512], FP, tag="scb", name="scB")

            def slot_view(slot, bw):
                if slot == 0:
                    return scA[:, :, 0:bw]
                if slot == 1:
                    return scA[:, :, 512:512 + bw]
                return scB[:, :, 0:bw]

            # --- warmup junk (PE clock gate) while first DMAs land ---
            for _ in range(12):
                nc.tensor.matmul(scA[:, 0, 0:512], lhsT=warm[:, :P], rhs=warm[:],
                                 start=True, stop=True)

            # --- prefix: q/k projections of pair 0, ct-outer over 8 banks ---
            gA = gp.tile([P, 512], FP, tag="g", name="pre_q0")
            gB = gp.tile([P, 512], FP, tag="g", name="pre_q1")
            qdst = [gA[:, :], gB[:, :], scB[:, 0, :], scB[:, 1, :]]
            kdst = [scA[:, 0, 0:512], scA[:, 0, 512:1024],
                    scA[:, 1, 0:512], scA[:, 1, 512:1024]]
            for ct in range(NCT):
                for gi in range(4):
                    nc.tensor.matmul(
                        qdst[gi], lhsT=wqt[0][:, ct, :],
                        rhs=xT[:, ct, 512 * gi:512 * gi + 512],
                        start=(ct == 0), stop=(ct == NCT - 1))
                for gi in range(4):
                    nc.tensor.matmul(
                        kdst[gi], lhsT=wkt[0][:, ct, :],
                        rhs=xT[:, ct, 512 * gi:512 * gi + 512],
                        start=(ct == 0), stop=(ct == NCT - 1))
            for gi in range(4):
                nc.vector.tensor_copy(q_sl[:, 0, 512 * gi:512 * gi + 512], qdst[gi])
                nc.vector.tensor_copy(k_sl[:, 0, 512 * gi:512 * gi + 512], kdst[gi])

            # --- filler closures (each ~1.7us of PE work into one gp bank) ---
            def qk_group(pn, which, gi):
                def emit():
                    gt = gp.tile([P, 512], FP, tag="g", name=f"{which}{pn}_{gi}")
                    wt = wqt[pn] if which == "q" else wkt[pn]
                    dst = q_sl if which == "q" else k_sl
                    for ct in range(NCT):
                        nc.tensor.matmul(
                            gt[:], lhsT=wt[:, ct, :],
                            rhs=xT[:, ct, 512 * gi:512 * gi + 512],
                            start=(ct == 0), stop=(ct == NCT - 1))
                    nc.vector.tensor_copy(dst[:, pn % 2, 512 * gi:512 * gi + 512], gt[:])
                return emit

            def v_group(g):
                def emit():
                    gt = gp.tile([P, 512], FP, tag="g", name=f"v{g}")
                    for ct in range(NCT):
                        nc.tensor.matmul(
                            gt[:], lhsT=xT[:, ct, P * g:P * g + P],
                            rhs=wv[:, ct, :],
                            start=(ct == 0), stop=(ct == NCT - 1))
                    nc.vector.tensor_copy(v_sb[:, g, :], gt[:])
                return emit

            def proj_group(tt, nb):
                def emit():
                    gt = gp.tile([P, 512], FP, tag="g", name=f"pj{tt}_{nb}")
                    for pp in range(4):
                        nc.tensor.matmul(
                            gt[:], lhsT=ao[:, pp, P * tt:P * tt + P],
                            rhs=wpt[:, pp, 512 * nb:512 * nb + 512],
                            start=(pp == 0), stop=(pp == 3))
                    yt = ytp.tile([P, 512], BF, tag="yt", name=f"yt{tt}_{nb}")
                    nc.vector.tensor_copy(yt[:], gt[:])
                    nc.sync.dma_start(
                        y_d.ap()[P * tt:P * tt + P, 512 * nb:512 * nb + 512], yt[:])
                return emit

            # --- attention pairs ---
            for p in range(4):
                sl = p % 2
                if p == 0:
                    fill = [v_group(g) for g in range(NST)] \
                        + [qk_group(1, w, gi) for w in ("q", "k") for gi in range(4)]
                    stage_w(1)
                elif p < 3:
                    fill = [qk_group(p + 1, w, gi) for w in ("q", "k") for gi in range(4)]
                    stage_w(p + 1)
                else:
                    fill = []
                bcount = 0

                def pop_fill():
                    # p0/p3 have enough filler work to pop one per block;
                    # p1/p2 spread theirs every other block.  Junk matmuls
                    # keep the PE's clock gate warm once real fillers dry up.
                    if fill and (p in (0, 3) or bcount % 2 == 0):
                        n = 2 if (p == 3 and len(fill) > 8) else 1
                        for _ in range(min(n, len(fill))):
                            fill.pop(0)()
                    elif not fill and bcount % 2 == 0:
                        gt = gp.tile([P, 512], FP, tag="g", name=f"jk{p}_{bcount}")
                        for _ in range(4):
                            nc.tensor.matmul(gt[:], lhsT=warm[:, :P], rhs=warm[:],
                                             start=True, stop=True)

                def emit_av(c):
                    gt = gp.tile([P, 512], FP, tag="g", name=f"av{p}_{c}")
                    jmax = 4 * c + 3
                    for j in range(jmax + 1):
                        lo2 = max(512 * c, P * j)
                        for hl in range(2):
                            nc.tensor.matmul(
                                gt[64 * hl:64 * hl + 64, lo2 - 512 * c:512],
                                lhsT=vp[:, sl, j, 64 * hl:64 * hl + 64],
                                rhs=prow[j][:, hl, lo2 - P * j:512 * c + 512 - P * j],
                                start=(j == 0), stop=(j == jmax))
                    nc.vector.tensor_copy(ao[:, p, 512 * c:512 * c + 512], gt[:])
                    if p == 3:
                        for tt in range(4 * c, 4 * c + 4):
                            fill.append(proj_group(tt, 0))
                            fill.append(proj_group(tt, 1))

                def make_stats(i, z, zs, nb):
                    def emit():
                        for b in range(1, nb):
                            nc.gpsimd.tensor_add(zs[:, 0:1], zs[:, 0:1],
                                                 zs[:, b:b + 1])
                        nc.vector.reduce_sum(z[:, 0:1], prow[i][:, 0, :], axis=AX)
                        nc.gpsimd.tensor_sub(z[:, 1:2], zs[:, 0:1], z[:, 0:1])
                        rz = st.tile([P, 2], FP, tag="rz", name=f"rz{p}_{i}")
                        nc.vector.reciprocal(rz[:], z[:])
                        for hl in range(2):
                            nc.vector.tensor_scalar_mul(
                                vp[:, sl, i, 64 * hl:64 * hl + 64],
                                v_sb[:, i, 128 * p + 64 * hl:128 * p + 64 * hl + 64],
                                rz[:, hl:hl + 1])
                    return emit

                prow = {}
                pend_stats = None
                for i in range(NST):
                    t0 = P * i
                    w_i = T - t0
                    prow[i] = prp.tile([P, 2, w_i], BF, tag=f"pr{i}",
                                       name=f"pr{p}_{i}")
                    z = st.tile([P, 2], FP, tag="z", name=f"z{p}_{i}")
                    zs = st.tile([P, 3], FP, tag="zs", name=f"zs{p}_{i}")
                    blks = blocks(i)
                    for b, (lo, hi, slot) in enumerate(blks):
                        bw = hi - lo
                        sc = slot_view(slot, bw)
                        for clo in range(0, bw, 512):
                            chi = min(clo + 512, bw)
                            for hl in range(2):
                                hb = 64 * hl
                                nc.tensor.matmul(
                                    sc[:, hl, clo:chi],
                                    lhsT=k_sl[hb:hb + 64, sl, t0:t0 + P],
                                    rhs=q_sl[hb:hb + 64, sl, lo + clo:lo + chi],
                                    start=True, stop=True)
                        if b == 0:
                            # causal mask: additive -1e30 triangle on the
                            # diagonal 128-block (block 0 always starts at t0)
                            nc.vector.tensor_add(sc[:, :, 0:128], sc[:, :, 0:128],
                                                 trif[:])
                        # merged two-head exp; accumulator = zh0+zh1 per
                        # partition (both heads share s on a partition)
                        nc.scalar.activation(
                            prow[i][:, :, lo - t0:hi - t0], sc[:],
                            EXP, scale=SCALE, accum_out=zs[:, b:b + 1])
                        pop_fill()
                        bcount += 1
                    # stats of the previous tile: emitted after this tile's
                    # score blocks so the DVE reduce doesn't queue ahead of
                    # the trif-add gating this tile's first exp
                    if pend_stats is not None:
                        pend_stats()
                    pend_stats = make_stats(i, z, zs, len(blks))
                    if i in (4, 8, 12):
                        emit_av(i // 4 - 1)
                pend_stats()
                emit_av(3)
                for f in fill:
                    f()

    nc.compile()
    return nc


def _get_nc():
    if "nc" not in _CACHE:
        _CACHE["nc"] = _build_nc()
    return _CACHE["nc"]


def _in_maps(x, Wq, Wk, Wv, Wp):
    import ml_dtypes
    trif = np.tril(np.full((P, P), NEG, np.float32), -1)
    trif2 = np.concatenate([trif, trif], 1)
    maps = []
    for b in range(B):
        xT = np.ascontiguousarray(x[b].T)
        for g in range(2):
            heads = range(8 * g, 8 * g + 8)
            maps.append({
                "xt": xT.astype(ml_dtypes.bfloat16),
                "wq": np.ascontiguousarray(np.concatenate([Wq[h] for h in heads], 1)).astype(ml_dtypes.bfloat16),
                "wk": np.ascontiguousarray(np.concatenate([Wk[h] for h in heads], 1)).astype(ml_dtypes.bfloat16),
                "wv": np.ascontiguousarray(np.concatenate([Wv[h] for h in heads], 1)).astype(ml_dtypes.bfloat16),
                "wpt": np.ascontiguousarray(Wp[:, 512 * g:512 * g + 512].T).astype(ml_dtypes.bfloat16),
                "trif": np.ascontiguousarray(trif2),
            })
    return maps


def kernel(x, Wq, Wk, Wv, Wp, bp):
    from concourse.bass_utils import run_bass_kernel_spmd

    x = np.asarray(x, np.float32)
    Wq = np.asarray(Wq, np.float32)
    Wk = np.asarray(Wk, np.float32)
    Wv = np.asarray(Wv, np.float32)
    Wp = np.asarray(Wp, np.float32)
    bp = np.asarray(bp, np.float32)

    nc = _get_nc()
    res = run_bass_kernel_spmd(nc, _in_maps(x, Wq, Wk, Wv, Wp), list(range(8)))
    y = np.empty((B, T, C), np.float32)
    for b in range(B):
        y[b] = (res.results[2 * b]["y"].astype(np.float32)
                + res.results[2 * b + 1]["y"].astype(np.float32) + bp)
    return y


# revision 18
# speedup vs baseline: 1.1496x; 1.0382x over previous
"""Multi-head attention (query-axis softmax variant) on 8 Trainium2 NeuronCores.

Problem: B=4, T=2048, C=1024, H=16, Dh=64.
  q/k/v = per-head projections of x; wei = (q k^T) * C**-0.5, causal-masked;
  softmax over the QUERY axis (axis=2 of (B,H,T,S)); out = attn @ v, concat
  heads, project with Wp and add bp.

Sharding: 8 cores = 4 batches x 2 head-groups (8 heads each).  Each core
computes a partial projection output for its batch; host sums the two
group partials per batch and adds the bias.

Per-core dataflow is fully "transposed" (features on partitions, tokens on
the free axis) so the query-axis softmax stats become free-axis reductions.
Score blocks rotate through pipelined PSUM slots (A = [P,2,1024], its second
half, and B = [P,2,512]) so the next block's matmuls run while the previous
block's exp drains.  Exps are merged two-head 3D-AP ACTIVATEs whose
accumulator yields zh0+zh1 per partition (both heads share the same s on a
partition); h0's Z comes from one DVE reduce of the retained P row and h1's
by subtraction on gpsimd.  attout accumulates per 512-col chunk over the
retained P rows with col-group-paired matmuls; projections (v, q/k of the
next pair, output) fill the PE between score bursts.
"""
import numpy as np

T = 2048
C = 1024
H = 16
DH = 64
B = 4
SCALE = float(C) ** -0.5
NEG = -1e30
P = 128
NCT = 8       # c-tiles (contraction tiles of 128 over C)
NST = 16      # s-tiles of 128 over T

_CACHE = {}


def _build_nc():
    import concourse.bacc as bacc
    import concourse.tile as tile
    import concourse.mybir as mybir

    FP = mybir.dt.float32
    BF = mybir.dt.bfloat16
    AX = mybir.AxisListType.X
    EXP = mybir.ActivationFunctionType.Exp

    nc = bacc.Bacc("TRN2", target_bir_lowering=False, debug=False, num_devices=8)

    xT_d = nc.declare_dram_parameter("xt", [C, T], BF, isOutput=False)
    wq_d = nc.declare_dram_parameter("wq", [C, 512], BF, isOutput=False)
    wk_d = nc.declare_dram_parameter("wk", [C, 512], BF, isOutput=False)
    wv_d = nc.declare_dram_parameter("wv", [C, 512], BF, isOutput=False)
    wp_d = nc.declare_dram_parameter("wpt", [512, C], BF, isOutput=False)
    trif_d = nc.declare_dram_parameter("trif", [P, 256], FP, isOutput=False)
    y_d = nc.declare_dram_parameter("y", [T, C], BF, isOutput=True)

    def blocks(i):
        # all score blocks are <= 512 wide; they rotate through three
        # [P, 2, 512] psum slot units for pipeline depth 3 between a block's
        # exp and later blocks' matmuls
        t0 = P * i
        out = []
        lo = t0
        while lo < T:
            hi = min(lo + 512 - lo % 512 if lo % 512 else lo + 512, T)
            out.append((lo, hi))
            lo = hi
        return out

    with tile.TileContext(nc) as tc:
        with (
            tc.tile_pool(name="perm", bufs=1) as perm,
            tc.tile_pool(name="wst", bufs=2) as wst,
            tc.tile_pool(name="prp", bufs=1) as prp,
            tc.tile_pool(name="st", bufs=6) as st,
            tc.tile_pool(name="ytp", bufs=2) as ytp,
            tc.tile_pool(name="sca", bufs=1, space="PSUM") as sca,
            tc.tile_pool(name="scb", bufs=1, space="PSUM") as scb,
            tc.tile_pool(name="gp", bufs=2, space="PSUM") as gp,
        ):
            trif = perm.tile([P, 2, 128], FP, tag="trif")
            nc.sync.dma_start(trif[:], trif_d.ap().rearrange("p (a c) -> p a c", a=2))

            warm = perm.tile([P, 512], BF, tag="warm")
            nc.vector.memset(warm[:], 0.0)
            dummy = perm.tile([P, 1], BF, tag="dummy")
            # preload the exp table set off the critical path
            nc.scalar.activation(dummy[:], warm[:, 0:1], EXP, scale=1.0)

            wqt = {}
            wkt = {}

            def stage_w(pn):
                wqt[pn] = wst.tile([P, NCT, P], BF, tag="wq", name=f"wq{pn}")
                wkt[pn] = wst.tile([P, NCT, P], BF, tag="wk", name=f"wk{pn}")
                nc.sync.dma_start(
                    wqt[pn][:],
                    wq_d.ap()[:, P * pn:P * pn + P].rearrange("(a c) m -> c a m", c=P))
                nc.sync.dma_start(
                    wkt[pn][:],
                    wk_d.ap()[:, P * pn:P * pn + P].rearrange("(a c) m -> c a m", c=P))

            stage_w(0)
            xT = perm.tile([P, NCT, T], BF, tag="xT")
            for ct in range(NCT):
                nc.sync.dma_start(xT[:, ct, :], xT_d.ap()[P * ct:P * ct + P, :])
            wv = perm.tile([P, NCT, 512], BF, tag="wv")
            nc.sync.dma_start(wv[:], wv_d.ap().rearrange("(a c) m -> c a m", c=P))
            wpt = perm.tile([P, 4, C], BF, tag="wpt")
            nc.sync.dma_start(wpt[:], wp_d.ap().rearrange("(a c) m -> c a m", c=P))

            q_sl = perm.tile([P, 2, T], BF, tag="q")
            k_sl = perm.tile([P, 2, T], BF, tag="k")
            v_sb = perm.tile([P, NST, 512], BF, tag="v")
            ao = perm.tile([P, 4, T], BF, tag="ao")
            vp = perm.tile([P, 2, NST, 128], BF, tag="vp")

            # score slots: allocated once, rotation handled by subtile deps
            scA = sca.tile([P, 2, 1024], FP, tag="sca", name="scA")
            scB = scb.tile([P, 2, 512], FP, tag="scb", name="scB")

            def slot_view(slot, bw):
                if slot == 0:
                    return scA[:, :, 0:bw]
                if slot == 1:
                    return scA[:, :, 512:512 + bw]
                return scB[:, :, 0:bw]

            slot_cur = [0]

            def next_slot(bw):
                s = slot_cur[0]
                slot_cur[0] = (s + 1) % 3
                return slot_view(s, bw)

            # --- warmup junk (PE clock gate) while first DMAs land ---
            for _ in range(12):
                nc.tensor.matmul(scA[:, 0, 0:512], lhsT=warm[:, :P], rhs=warm[:],
                                 start=True, stop=True)

            # --- prefix: q of pair 0 (all 4 groups) + k group 0 only; the
            # remaining k groups become pair-0's first fillers (k group g is
            # first needed as lhsT at s-tile 4g)
            gA = gp.tile([P, 512], FP, tag="g", name="pre_q0")
            gB = gp.tile([P, 512], FP, tag="g", name="pre_q1")
            qdst = [gA[:, :], gB[:, :], scB[:, 0, :], scB[:, 1, :]]
            kdst0 = scA[:, 0, 0:512]
            for ct in range(NCT):
                for gi in range(4):
                    nc.tensor.matmul(
                        qdst[gi], lhsT=wqt[0][:, ct, :],
                        rhs=xT[:, ct, 512 * gi:512 * gi + 512],
                        start=(ct == 0), stop=(ct == NCT - 1))
                nc.tensor.matmul(
                    kdst0, lhsT=wkt[0][:, ct, :], rhs=xT[:, ct, 0:512],
                    start=(ct == 0), stop=(ct == NCT - 1))
            for gi in range(4):
                nc.vector.tensor_copy(q_sl[:, 0, 512 * gi:512 * gi + 512], qdst[gi])
            nc.vector.tensor_copy(k_sl[:, 0, 0:512], kdst0)

            # --- filler closures (each ~1.7us of PE work into one gp bank) ---
            def qk_group(pn, which, gi):
                def emit():
                    gt = gp.tile([P, 512], FP, tag="g", name=f"{which}{pn}_{gi}")
                    wt = wqt[pn] if which == "q" else wkt[pn]
                    dst = q_sl if which == "q" else k_sl
                    for ct in range(NCT):
                        nc.tensor.matmul(
                            gt[:], lhsT=wt[:, ct, :],
                            rhs=xT[:, ct, 512 * gi:512 * gi + 512],
                            start=(ct == 0), stop=(ct == NCT - 1))
                    nc.vector.tensor_copy(dst[:, pn % 2, 512 * gi:512 * gi + 512], gt[:])
                return emit

            def v_group(g):
                def emit():
                    gt = gp.tile([P, 512], FP, tag="g", name=f"v{g}")
                    for ct in range(NCT):
                        nc.tensor.matmul(
                            gt[:], lhsT=xT[:, ct, P * g:P * g + P],
                            rhs=wv[:, ct, :],
                            start=(ct == 0), stop=(ct == NCT - 1))
                    nc.vector.tensor_copy(v_sb[:, g, :], gt[:])
                return emit

            def proj_group(tt, nb):
                def emit():
                    gt = gp.tile([P, 512], FP, tag="g", name=f"pj{tt}_{nb}")
                    for pp in range(4):
                        nc.tensor.matmul(
                            gt[:], lhsT=ao[:, pp, P * tt:P * tt + P],
                            rhs=wpt[:, pp, 512 * nb:512 * nb + 512],
                            start=(pp == 0), stop=(pp == 3))
                    yt = ytp.tile([P, 512], BF, tag="yt", name=f"yt{tt}_{nb}")
                    nc.vector.tensor_copy(yt[:], gt[:])
                    nc.sync.dma_start(
                        y_d.ap()[P * tt:P * tt + P, 512 * nb:512 * nb + 512], yt[:])
                return emit

            # --- attention pairs ---
            for p in range(4):
                sl = p % 2
                if p == 0:
                    fill = [qk_group(0, "k", gi) for gi in range(1, 4)] \
                        + [v_group(g) for g in range(NST)] \
                        + [qk_group(1, w, gi) for w in ("q", "k") for gi in range(4)]
                    stage_w(1)
                elif p < 3:
                    fill = [qk_group(p + 1, w, gi) for w in ("q", "k") for gi in range(4)]
                    stage_w(p + 1)
                else:
                    fill = []
                bcount = 0

                def pop_fill():
                    # p0/p3 have enough filler work to pop one per block;
                    # p1/p2 spread theirs every other block.  Junk matmuls
                    # keep the PE's clock gate warm once real fillers dry up.
                    if fill and (p in (0, 3) or bcount % 2 == 0):
                        n = 2 if (p == 3 and len(fill) > 4) else 1
                        for _ in range(min(n, len(fill))):
                            fill.pop(0)()
                    elif not fill and bcount % 2 == 0:
                        gt = gp.tile([P, 512], FP, tag="g", name=f"jk{p}_{bcount}")
                        for _ in range(4):
                            nc.tensor.matmul(gt[:], lhsT=warm[:, :P], rhs=warm[:],
                                             start=True, stop=True)

                def emit_av(clo, chi, jmax, tts):
                    cw = chi - clo
                    gt = gp.tile([P, 512], FP, tag="g", name=f"av{p}_{clo}")
                    for j in range(jmax + 1):
                        lo2 = max(clo, P * j)
                        for hl in range(2):
                            nc.tensor.matmul(
                                gt[64 * hl:64 * hl + 64, lo2 - clo:cw],
                                lhsT=vp[:, sl, j, 64 * hl:64 * hl + 64],
                                rhs=prow[j][:, hl, lo2 - P * j:chi - P * j],
                                start=(j == 0), stop=(j == jmax))
                    nc.vector.tensor_copy(ao[:, p, clo:chi], gt[:, 0:cw])
                    if p == 3:
                        for tt in tts:
                            fill.append(proj_group(tt, 0))
                            fill.append(proj_group(tt, 1))

                def make_stats(i, z, zs, nb):
                    def emit():
                        for b in range(1, nb):
                            nc.gpsimd.tensor_add(zs[:, 0:1], zs[:, 0:1],
                                                 zs[:, b:b + 1])
                        nc.vector.reduce_sum(z[:, 0:1], prow[i][:, 0, :], axis=AX)
                        nc.gpsimd.tensor_sub(z[:, 1:2], zs[:, 0:1], z[:, 0:1])
                        rz = st.tile([P, 2], FP, tag="rz", name=f"rz{p}_{i}")
                        nc.vector.reciprocal(rz[:], z[:])
                        for hl in range(2):
                            nc.vector.tensor_scalar_mul(
                                vp[:, sl, i, 64 * hl:64 * hl + 64],
                                v_sb[:, i, 128 * p + 64 * hl:128 * p + 64 * hl + 64],
                                rz[:, hl:hl + 1])
                    return emit

                prow = {}
                pend_stats = None
                for i in range(NST):
                    t0 = P * i
                    w_i = T - t0
                    prow[i] = prp.tile([P, 2, w_i], BF, tag=f"pr{i}",
                                       name=f"pr{p}_{i}")
                    z = st.tile([P, 2], FP, tag="z", name=f"z{p}_{i}")
                    zs = st.tile([P, 4], FP, tag="zs", name=f"zs{p}_{i}")
                    blks = blocks(i)
                    for b, (lo, hi) in enumerate(blks):
                        bw = hi - lo
                        sc = next_slot(bw)
                        for hl in range(2):
                            hb = 64 * hl
                            nc.tensor.matmul(
                                sc[:, hl, 0:bw],
                                lhsT=k_sl[hb:hb + 64, sl, t0:t0 + P],
                                rhs=q_sl[hb:hb + 64, sl, lo:hi],
                                start=True, stop=True)
                        if b == 0:
                            # causal mask: additive -1e30 triangle on the
                            # diagonal 128-block (block 0 always starts at t0)
                            nc.vector.tensor_add(sc[:, :, 0:128], sc[:, :, 0:128],
                                                 trif[:])
                        # merged two-head exp; accumulator = zh0+zh1 per
                        # partition (both heads share s on a partition)
                        nc.scalar.activation(
                            prow[i][:, :, lo - t0:hi - t0], sc[:],
                            EXP, scale=SCALE, accum_out=zs[:, b:b + 1])
                        pop_fill()
                        bcount += 1
                    # stats of the previous tile: emitted after this tile's
                    # score blocks so the DVE reduce doesn't queue ahead of
                    # the trif-add gating this tile's first exp
                    if pend_stats is not None:
                        pend_stats()
                    pend_stats = make_stats(i, z, zs, len(blks))
                    if i in (4, 8, 12):
                        c = i // 4 - 1
                        emit_av(512 * c, 512 * c + 512, 4 * c + 3,
                                range(4 * c, 4 * c + 4))
                    elif i == 14 and p == 3:
                        # early half of the last chunk so the output
                        # projection tail shrinks
                        emit_av(1536, 1792, 13, (12, 13))
                pend_stats()
                if p == 3:
                    emit_av(1792, 2048, 15, (14, 15))
                else:
                    emit_av(1536, 2048, 15, ())
                for f in fill:
                    f()

    nc.compile()
    return nc


def _get_nc():
    if "nc" not in _CACHE:
        _CACHE["nc"] = _build_nc()
    return _CACHE["nc"]


def _in_maps(x, Wq, Wk, Wv, Wp):
    import ml_dtypes
    trif = np.tril(np.full((P, P), NEG, np.float32), -1)
    trif2 = np.concatenate([trif, trif], 1)
    maps = []
    for b in range(B):
        xT = np.ascontiguousarray(x[b].T)
        for g in range(2):
            heads = range(8 * g, 8 * g + 8)
            maps.append({
                "xt": xT.astype(ml_dtypes.bfloat16),
                "wq": np.ascontiguousarray(np.concatenate([Wq[h] for h in heads], 1)).astype(ml_dtypes.bfloat16),
                "wk": np.ascontiguousarray(np.concatenate([Wk[h] for h in heads], 1)).astype(ml_dtypes.bfloat16),
                "wv": np.ascontiguousarray(np.concatenate([Wv[h] for h in heads], 1)).astype(ml_dtypes.bfloat16),
                "wpt": np.ascontiguousarray(Wp[:, 512 * g:512 * g + 512].T).astype(ml_dtypes.bfloat16),
                "trif": np.ascontiguousarray(trif2),
            })
    return maps


def kernel(x, Wq, Wk, Wv, Wp, bp):
    from concourse.bass_utils import run_bass_kernel_spmd

    x = np.asarray(x, np.float32)
    Wq = np.asarray(Wq, np.float32)
    Wk = np.asarray(Wk, np.float32)
    Wv = np.asarray(Wv, np.float32)
    Wp = np.asarray(Wp, np.float32)
    bp = np.asarray(bp, np.float32)

    nc = _get_nc()
    res = run_bass_kernel_spmd(nc, _in_maps(x, Wq, Wk, Wv, Wp), list(range(8)))
    y = np.empty((B, T, C), np.float32)
    for b in range(B):
        y[b] = (res.results[2 * b]["y"].astype(np.float32)
                + res.results[2 * b + 1]["y"].astype(np.float32) + bp)
    return y
